# revision 112
# baseline (speedup 1.0000x reference)
"""Causal multi-head attention on 8 Trainium2 NeuronCores.

Sharding: core c -> (batch g = c // 4, head-group p = c % 4, heads 4p..4p+3).
Each core projects Q/K/V for its batch with its 256 feature columns
(column-sharded w_q/w_k/w_v), runs causal attention for its 4 heads in
transposed (scores.T) layout with an augmented-ones column on V to get the
softmax denominators for free, computes the partial output projection with
its 256 rows of w_o, and a ReduceScatter over each batch group sums the
partials and hands every core its own 512-row output shard.

x / w_qkv / probs / V / masks in bf16 (halves DMA + 2x DVE mask-muls);
scores and all PSUM accumulation fp32; qT/kT/ctx/w_o float32r.
V is projected directly into natural [kpos, dim] layout (x chunk as the
stationary operand) so no PE transposes are needed. Right causal edge is
trimmed to 256-col granularity. Output-projection bias adds run on the
(otherwise idle) Pool engine.
"""

import numpy as np

B, S, D, H = 2, 2048, 1024, 16
DK = D // H  # 64
N_CORES = 8
FPC = 256  # features per core

_CACHE = {}

TRIM = (0, 128, 256, 256)  # leading fully-masked cols to skip, by ki%4


def _build_nc():
    import os as os_mod  # noqa: F401  (used in emit_oproj_half closure)
    import concourse.mybir as mybir
    import concourse.tile as tile
    from concourse import bacc

    F32 = mybir.dt.float32
    F32R = mybir.dt.float32r
    BF16 = mybir.dt.bfloat16
    Exp = mybir.ActivationFunctionType.Exp

    nc = bacc.Bacc("TRN2", target_bir_lowering=False, debug=False, num_devices=8)

    xq = nc.dram_tensor("xq", [D, S], BF16, kind="ExternalInput")
    xk = nc.dram_tensor("xk", [D, S], BF16, kind="ExternalInput")
    xv = nc.dram_tensor("xv", [D, S], BF16, kind="ExternalInput")
    wq = nc.dram_tensor("wq", [D, FPC], BF16, kind="ExternalInput")
    wk = nc.dram_tensor("wk", [D, FPC], BF16, kind="ExternalInput")
    wv = nc.dram_tensor("wv", [D, FPC], BF16, kind="ExternalInput")
    wo = nc.dram_tensor("wo", [FPC, D], F32, kind="ExternalInput")
    bq = nc.dram_tensor("bq", [FPC, 1], F32, kind="ExternalInput")
    bk = nc.dram_tensor("bk", [FPC, 1], F32, kind="ExternalInput")
    bv = nc.dram_tensor("bv", [1, FPC], F32, kind="ExternalInput")
    bo4 = nc.dram_tensor("bo4", [1, D], BF16, kind="ExternalInput")
    masks = nc.dram_tensor("masks", [128, 2048], BF16, kind="ExternalInput")
    out = nc.dram_tensor("out", [512, D], F32, kind="ExternalOutput")
    dbg = bool(os_mod.environ.get("BASS_DEBUG_DUMP"))
    if dbg:
        dq = nc.dram_tensor("dq", [2, 128, S], F32, kind="ExternalOutput")
        dk = nc.dram_tensor("dk", [2, 128, S], F32, kind="ExternalOutput")
        dv = nc.dram_tensor("dv", [128, 16, 260], BF16, kind="ExternalOutput")
        dctx = nc.dram_tensor("dctx", [2, 128, S], F32, kind="ExternalOutput")

    NQB = S // 512  # 4 q blocks

    from contextlib import ExitStack
    stack = ExitStack()
    with tile.TileContext(nc) as tc:
        with (
            tc.tile_pool(name="consts", bufs=1) as consts,
            tc.tile_pool(name="persist", bufs=1) as persist,
            tc.tile_pool(name="xin", bufs=6) as xin,
            tc.tile_pool(name="xkres", bufs=8) as xkres,
            tc.tile_pool(name="probs", bufs=4) as probs,
            tc.tile_pool(name="small", bufs=4) as small,
            tc.tile_pool(name="oout", bufs=4) as oout,
            tc.tile_pool(name="dram", bufs=1, space="DRAM") as dram,
        ):
            # ---- constants ----
            wq_s = consts.tile([128, 8, FPC], BF16, tag="wq")
            wk_s = consts.tile([128, 8, FPC], BF16, tag="wk")
            wv_s = consts.tile([128, 8, FPC], BF16, tag="wv")
            wo_s = consts.tile([128, 2, D], F32R, tag="wo")
            masks_s = consts.tile([128, 2048], BF16, tag="masks")
            bq_s = consts.tile([128, 2], F32, tag="bq")
            bk_s = consts.tile([128, 2], F32, tag="bk")
            bv_row = consts.tile([1, FPC], F32, tag="bvr")
            bv_bc = consts.tile([128, FPC], F32, tag="bvb")
            bo4_s = consts.tile([1, D], BF16, tag="bo4")
            ones_r = consts.tile([1, 128], BF16, tag="ones")
            wu_in = consts.tile([128, 64], BF16, tag="wu")

            # ---- persistent activations ----
            qT_s = [persist.tile([128, S], F32R, tag=f"qT{i}", name=f"qT{i}") for i in range(2)]
            kT_s = [persist.tile([128, S], F32R, tag=f"kT{i}", name=f"kT{i}") for i in range(2)]
            # V natural layout: [kpos(128) , s-tile st, head strips of 65 (64 dims + ones)]
            v_s = persist.tile([128, 16, 4 * 65], BF16, tag="v")
            ctx_s = [persist.tile([128, S], F32R, tag=f"ctx{i}", name=f"ctx{i}") for i in range(2)]

            # PE p-state: the cost model runs matmuls at full clock only when
            # their dispatch lands >3us after the engine's current busy-run
            # began. Seed one long busy run with cheap warm-up matmuls and
            # avoid engine idle gaps from here on.
            nc.vector.memset(wu_in[:], 0.5)
            nc.vector.memset(ones_r[:], 1.0)

            # startup DMAs: all on the sync queue so the shared DMA engine
            # serves them in exactly this order (x chunks just-in-time,
            # weights squeezed between)
            nc.sync.dma_start(wv_s[:, 0, :], wv[0:128, :])
            xt0 = xin.tile([128, S], BF16, tag="x", name="xt0")
            for piece in range(4):
                # parallel dispatch queues: each dma_start costs ~650ns of
                # queue-dispatch latency, serial per engine
                (nc.scalar if piece % 2 else nc.sync).dma_start(
                    xt0[:, 512 * piece : 512 * (piece + 1)],
                    xv[0:128, 512 * piece : 512 * (piece + 1)])
            nc.scalar.dma_start(bv_row[:], bv.ap())
            nc.scalar.dma_start(
                wv_s[:, 1:8, :],
                wv[128:1024, :].rearrange("(kc p) f -> p kc f", p=128))
            nc.gpsimd.partition_broadcast(bv_bc[:], bv_row[:])
            # ones columns of V_aug (col 64 of each head's 65-wide strip)
            for h in range(4):
                nc.vector.memset(v_s[:, :, 65 * h + 64 : 65 * h + 65].bitcast(BF16), 1.0)

            # ---- phase 1: projections (shared PSUM pool, slots recycle) ----
            with tc.tile_pool(name="psProj", bufs=8, space="PSUM") as psP:
                # V pass first (direct natural layout: x chunk slices stationary,
                # wv moving). psum tile sp covers s-tiles 2sp, 2sp+1 (256 feats each)
                psv = {sp: psP.tile([128, 512], F32, tag="pp", name=f"psv{sp}")
                       for sp in range(8)}
                def warm(n):  # PE warm-up / bridge runs (overwritten later)
                    for _ in range(n):
                        nc.tensor.matmul(
                            psv[7][0:64, 0:64], wu_in[:, 0:64], wu_in[:, 0:64],
                            start=True, stop=True, skip_group_check=True)
                warm(56)
                # pre-issue all chunk DMAs: the xin pool rotation turns this
                # into a bufs-deep prefetch pipeline
                xts = [xt0]
                for kc in range(1, 8):
                    xt = xin.tile([128, S], BF16, tag="x")
                    nc.sync.dma_start(xt[:], xv[128 * kc : 128 * (kc + 1), :])
                    xts.append(xt)
                for kc in range(8):
                    xt = xts[kc]
                    for sp in range(8):
                        for half in range(2):
                            st = 2 * sp + half
                            # one accumulation group per PSUM bank: start=True
                            # zeroes the whole bank, so only the first write
                            # into the bank may carry it
                            nc.tensor.matmul(
                                psv[sp][:, 256 * half : 256 * (half + 1)],
                                xt[:, 128 * st : 128 * (st + 1)],
                                wv_s[:, kc, :],
                                start=(kc == 0 and half == 0),
                                stop=(kc == 7 and half == 1),
                                skip_group_check=True,
                            )
                    if kc == 3:
                        # prefetch Q weights/bias + first xq chunk during V
                        nc.scalar.dma_start(wq_s[:], wq.ap().rearrange("(kc p) f -> p kc f", p=128))
                        nc.scalar.dma_start(bq_s[:], bq.ap().rearrange("(t p) o -> p (t o)", p=128))
                    elif kc == 5:
                        xq0 = xin.tile([128, S], BF16, tag="x", name="xq0")
                        nc.sync.dma_start(xq0[:], xq[0:128, :])
                    elif kc == 6:
                        nc.scalar.dma_start(wk_s[:], wk.ap().rearrange("(kc p) f -> p kc f", p=128))
                        nc.scalar.dma_start(bk_s[:], bk.ap().rearrange("(t p) o -> p (t o)", p=128))
                        nc.scalar.dma_start(masks_s[:], masks.ap())
                for sp in range(8):
                    for half in range(2):
                        st = 2 * sp + half
                        dst = v_s[:, st, :].rearrange("p (h x) -> p h x", x=65)[:, :, 0:64]
                        nc.vector.tensor_add(
                            dst,
                            psv[sp][:, 256 * half : 256 * (half + 1)].rearrange(
                                "p (h x) -> p h x", x=64),
                            bv_bc[:].rearrange("p (h x) -> p h x", x=64),
                        )

                # Q pass: psum [2pt x 4qb] accumulate over 8 k-chunks
                psq = {(pt, qb): psP.tile([128, 512], F32, tag="pp", name=f"psq{pt}{qb}")
                       for pt in range(2) for qb in range(NQB)}
                xts = [xq0]
                for kc in range(1, 8):
                    xt = xin.tile([128, S], BF16, tag="x")
                    nc.sync.dma_start(xt[:], xq[128 * kc : 128 * (kc + 1), :])
                    xts.append(xt)
                for kc in range(8):
                    xt = xts[kc]
                    for pt in range(2):
                        for qb in range(NQB):
                            nc.tensor.matmul(
                                psq[(pt, qb)][:],
                                wq_s[:, kc, 128 * pt : 128 * (pt + 1)],
                                xt[:, 512 * qb : 512 * (qb + 1)],
                                start=(kc == 0), stop=(kc == 7),
                            )
                    if kc == 5:
                        xk0 = xkres.tile([128, S], BF16, tag="xk", name="xk0")
                        nc.sync.dma_start(xk0[:], xk[0:128, :])
                for pt in range(2):
                    for qb in range(NQB):
                        nc.vector.tensor_scalar_add(
                            qT_s[pt][:, 512 * qb : 512 * (qb + 1)],
                            psq[(pt, qb)][:], bq_s[:, pt : pt + 1],
                        )
                # K pass
                psk = {(0, qb): psP.tile([128, 512], F32, tag="pp", name=f"psk0{qb}")
                       for qb in range(NQB)}
                xts = [xk0]
                for kc in range(1, 8):
                    xt = xkres.tile([128, S], BF16, tag="xk")
                    nc.sync.dma_start(xt[:], xk[128 * kc : 128 * (kc + 1), :])
                    xts.append(xt)
                xk_res = xts
                for kc in range(8):
                    xt = xts[kc]
                    for qb in range(NQB):
                        nc.tensor.matmul(
                            psk[(0, qb)][:],
                            wk_s[:, kc, 0:128],
                            xt[:, 512 * qb : 512 * (qb + 1)],
                            start=(kc == 0), stop=(kc == 7),
                        )

                # spread the last copies across 3 engines: the attention
                # PSUM pools can only open once every psProj tile is released
                for qb in range(NQB):
                    dst = kT_s[0][:, 512 * qb : 512 * (qb + 1)]
                    if qb % 2 == 1:
                        nc.scalar.activation(
                            out=dst, in_=psk[(0, qb)][:],
                            func=mybir.ActivationFunctionType.Identity,
                            bias=bk_s[:, 0:1])
                    else:
                        nc.vector.tensor_scalar_add(
                            dst, psk[(0, qb)][:], bk_s[:, 0:1])

            if dbg:
                for i in range(2):
                    nc.sync.dma_start(dq[i, :, :], qT_s[i][:].bitcast(F32))
                    nc.sync.dma_start(dk[i, :, :], kT_s[i][:].bitcast(F32))
                nc.sync.dma_start(dv[:, :, :], v_s[:])

            # ---- output projection halves + split ReduceScatter ----
            rs_in = [dram.tile([S // 2, D], F32, name=f"rs_in{i}") for i in range(2)]
            rs_out = [dram.tile([256, D], F32, name=f"rs_out{i}") for i in range(2)]

            def emit_oproj_half(half, psO):
                # q rows [1024*half, 1024*half+1024) = ctx_s cols of qb-pair `half`
                # bias folded in via a ones-row matmul; rs_in DMA'd straight
                # from PSUM
                for sl in range(8):
                    st = 8 * half + sl
                    po = psO.tile([128, 2, 512], F32, tag="po", bufs=4, name="po")
                    for nb in range(2):
                        for fc in range(2):
                            nc.tensor.matmul(
                                po[:, nb, :],
                                ctx_s[fc][:, 128 * st : 128 * (st + 1)],
                                wo_s[:, fc, 512 * nb : 512 * (nb + 1)],
                                start=(fc == 0), stop=False,
                                skip_group_check=True,
                            )
                        nc.tensor.matmul(
                            po[:, nb, :],
                            ones_r[:],
                            bo4_s[:, 512 * nb : 512 * (nb + 1)],
                            start=False, stop=True,
                            skip_group_check=True,
                        )
                    ot = oout.tile([128, D], F32, tag="ot")
                    if sl % 2 == 0:
                        nc.scalar.activation(
                            out=ot[:].rearrange("p (n x) -> p n x", n=2), in_=po[:],
                            func=mybir.ActivationFunctionType.Copy)
                    else:
                        nc.vector.tensor_copy(
                            ot[:].rearrange("p (n x) -> p n x", n=2), po[:])
                    nc.sync.dma_start(rs_in[half][128 * sl : 128 * (sl + 1), :], ot[:])
                if not os_mod.environ.get("BASS_SIM_NO_RS"):
                    import concourse.mybir as mybir_mod
                    nc.gpsimd.collective_compute(
                        "ReduceScatter", mybir_mod.AluOpType.add,
                        replica_groups=[[0, 1, 2, 3], [4, 5, 6, 7]],
                        ins=[rs_in[half].opt()], outs=[rs_out[half].opt()],
                    )
                    nc.sync.dma_start(
                        out[256 * half : 256 * (half + 1), :], rs_out[half][:])
                else:
                    nc.sync.dma_start(
                        out[256 * half : 256 * (half + 1), :],
                        rs_in[half][0:256, :])

            # ---- phase 2: attention ----
            # One head-stream at a time; PSUM split 4/2/2 banks:
            #   psS: sc scores [128,1024] x2 bufs (4 banks)
            #   psA: ctx accumulators [65,1024] x1 buf (2 banks)
            #   psO: oproj po [128,2,512] x1 buf (2 banks)
            # A depth-2 software pipeline (ctx matmuls for step ki-2 emitted
            # after the score matmuls for step ki) hides both the exp/mask
            # chain and the ctx-slot norm drain at head seams. Output
            # projection tiles are stitched one-at-a-time into the next
            # qb-pair's (Act-bound) attention loop.
            first = True
            with (
                tc.tile_pool(name="rbcp", bufs=2) as rbcp,
                tc.tile_pool(name="psS", bufs=1, space="PSUM") as psS,
                tc.tile_pool(name="psA", bufs=1, space="PSUM") as psA,
                tc.tile_pool(name="psO", bufs=1, space="PSUM") as psO,
            ):
                def emit_po_st(st, half, alt=False, tail=False):
                    if alt:
                        pot = psS.tile([128, 1024], F32, tag="sc", bufs=2, name="sc")
                        po = pot[:].rearrange("p (n x) -> p n x", n=2)
                    else:
                        pot = psO.tile([128, 2, 512], F32, tag="po", bufs=1, name="po")
                        po = pot[:]
                    for nb in range(2):
                        for fc in range(2):
                            nc.tensor.matmul(
                                po[:, nb, :],
                                ctx_s[fc][:, 128 * st : 128 * (st + 1)],
                                wo_s[:, fc, 512 * nb : 512 * (nb + 1)],
                                start=(fc == 0), stop=False,
                                skip_group_check=True,
                            )
                        nc.tensor.matmul(
                            po[:, nb, :], ones_r[:],
                            bo4_s[:, 512 * nb : 512 * (nb + 1)],
                            start=False, stop=True,
                            skip_group_check=True,
                        )
                    ot = oout.tile([128, D], F32, tag="ot")
                    otv = ot[:].rearrange("p (n x) -> p n x", n=2)
                    if alt or tail:
                        # tail: split the copy across Act+DVE (halves latency)
                        nc.scalar.activation(
                            out=otv[:, 0, :], in_=po[:, 0, :],
                            func=mybir.ActivationFunctionType.Copy)
                        nc.vector.tensor_copy(otv[:, 1, :], po[:, 1, :])
                    else:
                        # stitched into Act-bound attention: keep Act free
                        nc.vector.tensor_copy(otv, po[:])
                    sl = st % 8
                    nc.sync.dma_start(rs_in[half][128 * sl : 128 * (sl + 1), :], ot[:])
                    if sl < 2 and os_mod.environ.get("BASS_SIM_NO_RS"):
                        # NO_RS stub: emit the out rows straight from SBUF so
                        # they don't queue behind the tail rs_in writes
                        nc.sync.dma_start(
                            out[256 * half + 128 * sl : 256 * half + 128 * (sl + 1), :],
                            ot[:])

                stitch = []
                # K projection for heads 2-3 (kT_s[1]) runs inside the
                # Act-bound qbp0 attention, in borrowed oproj PSUM banks
                kbs = {}

                def kpt1_thunk(pair, half):
                    def f():
                        if pair not in kbs:
                            kbs[pair] = psO.tile([128, 2, 512], F32, tag="po",
                                                 bufs=1, name="po")
                        kb = kbs[pair]
                        for kc in range(4 * half, 4 * half + 4):
                            for j in range(2):
                                qb = 2 * pair + j
                                nc.tensor.matmul(
                                    kb[:, j, :], wk_s[:, kc, 128:256],
                                    xk_res[kc][:, 512 * qb : 512 * (qb + 1)],
                                    start=(kc == 0), stop=(kc == 7),
                                    skip_group_check=True)
                        if half == 1:
                            for j in range(2):
                                qb = 2 * pair + j
                                dst = kT_s[1][:, 512 * qb : 512 * (qb + 1)]
                                if j:
                                    nc.scalar.activation(
                                        out=dst, in_=kb[:, j, :],
                                        func=mybir.ActivationFunctionType.Identity,
                                        bias=bk_s[:, 1:2])
                                else:
                                    nc.vector.tensor_scalar_add(
                                        dst, kb[:, j, :], bk_s[:, 1:2])
                    return f

                bgwork = [kpt1_thunk(p, hf) for p in (0, 1) for hf in (0, 1)]
                for qbp in range(2):
                    nkt = 8 * qbp + 8  # k-tiles needed by this qb pair
                    q0, q1 = 2 * qbp, 2 * qbp + 1
                    step = 0
                    for h in range(4):
                        pt, row = h // 2, 64 * (h % 2)
                        qT_h = qT_s[pt][row : row + 64, :]
                        kT_h = kT_s[pt][row : row + 64, :]
                        ctx_t = psA.tile([65, 1024], F32, tag="ctx", bufs=1, name="ctx")

                        def norm_half(hf, qbp_=qbp, pt=pt, row=row, ctx_t=ctx_t):
                            cs = slice(512 * hf, 512 * (hf + 1))
                            recip = small.tile([1, 512], F32, tag="recip")
                            nc.vector.reciprocal(recip[:], ctx_t[64:65, cs])
                            rbc = rbcp.tile([64, 512], F32, tag="rbc", bufs=6)
                            nc.gpsimd.partition_broadcast(rbc[:], recip[:])
                            nc.vector.tensor_mul(
                                ctx_s[pt][row : row + 64,
                                          1024 * qbp_ + 512 * hf : 1024 * qbp_ + 512 * (hf + 1)],
                                ctx_t[0:64, cs], rbc[:],
                            )

                        pend = []

                        def pop_pend():
                            group, lo_stop = pend.pop(0)
                            for dst, vtl, src_, st_, sp_ in group:
                                nc.tensor.matmul(dst, vtl, src_, start=st_, stop=sp_,
                                                 skip_group_check=True)
                            return lo_stop

                        # single-qb tail k-tiles are packed two per sc tile
                        # (one exp instead of two, halves the step count)
                        items = ([[ki] for ki in range(min(nkt, 4 * q1))]
                                 + [[4 * q1, 4 * q1 + 1], [4 * q1 + 2, 4 * q1 + 3]])
                        step_in_head = 0
                        for item in items:
                            if qbp == 0 and step_in_head % 3 == 2 and not bgwork:
                                sc = psO.tile([128, 2, 512], F32, tag="po", bufs=1,
                                              name="po")[:].rearrange("p n x -> p (n x)")
                            else:
                                sc = psS.tile([128, 1024], F32, tag="sc", bufs=2, name="sc")
                            pr = probs.tile([128, 1024], BF16, tag="pr", bufs=8, name="pr")
                            paired = len(item) > 1
                            infos = []
                            exp_lo = None
                            for j, ki in enumerate(item):
                                dqb = ki // 4  # diagonal q block (absolute)
                                has_diag = dqb >= q0
                                qlo = max(dqb, q0)
                                off = TRIM[ki % 4] if has_diag else 0
                                infos.append((j, ki, has_diag, qlo, off))
                                # matmul writes stay within one PSUM bank
                                # (512 f32): one matmul per 512-col q block.
                                # paired k-tiles write the full bank so the
                                # shared exp never reads unwritten PSUM
                                for qb in range(qlo, q1 + 1):
                                    o = 0 if paired else (off if qb == qlo else 0)
                                    r = (512 * j if paired
                                         else 512 * (qb - 2 * qbp)) + o
                                    nc.tensor.matmul(
                                        sc[:, r : (r - o) + 512],
                                        kT_h[:, 128 * ki : 128 * (ki + 1)],
                                        qT_h[:, 512 * qb + o : 512 * (qb + 1)],
                                        start=True, stop=True,
                                    )
                                    if j == 0 and qb == qlo:
                                        exp_lo = r
                            nc.scalar.activation(
                                out=pr[:, exp_lo:1024], in_=sc[:, exp_lo:1024],
                                func=Exp, scale=0.125,
                            )
                            cur = []
                            lo_stop = False
                            for j, ki, has_diag, qlo, off in infos:
                                vt = v_s[:, ki, 65 * h : 65 * h + 65]
                                prm = None
                                if has_diag:
                                    pcol = (512 * j if paired
                                            else 512 * (qlo - 2 * qbp)) + off
                                    prm = probs.tile([128, 512], BF16, tag="prm", bufs=8, name="prm")
                                    nc.vector.tensor_mul(
                                        prm[:, 0 : 512 - off], pr[:, pcol : (pcol - off) + 512],
                                        masks_s[:, 512 * (ki % 4) + off : 512 * (ki % 4) + 512],
                                    )
                                for qb in range(qlo, q1 + 1):
                                    o = off if qb == qlo else 0
                                    pcol = (512 * j if paired
                                            else 512 * (qb - 2 * qbp)) + o
                                    ccol = 512 * (qb - 2 * qbp) + o
                                    if has_diag and qb == qlo:
                                        rhs = prm[:, 0 : 512 - o]
                                    else:
                                        rhs = pr[:, pcol : (pcol - o) + 512]
                                    cur.append((
                                        ctx_t[:, ccol : (ccol - o) + 512],
                                        vt, rhs,
                                        ki == 0, ki == 4 * qb + 3))
                                    if ki == 8 * qbp + 3 and qb == 2 * qbp:
                                        lo_stop = True
                            pend.append((cur, lo_stop))
                            depth = 2
                            while len(pend) > depth:
                                if pop_pend():
                                    # cols [0:512] of ctx are complete:
                                    # normalize the first half early
                                    norm_half(0)
                                    if qbp == 1 and h == 3:
                                        # oproj1 tiles reading half-0 cols are
                                        # now fully normalized: stitch them in
                                        stitch += [(8 + sl, 1) for sl in range(4)]
                            step += 1
                            step_in_head += 1
                            if bgwork and qbp == 0 and step >= 1:
                                bgwork.pop(0)()
                            if stitch and (step % 6 == 5
                                           or (qbp == 1 and h == 3
                                               and step_in_head >= 12)):
                                st_, hf_ = stitch.pop(0)
                                emit_po_st(st_, hf_)
                        while pend:
                            if pop_pend():
                                norm_half(0)
                                if qbp == 1 and h == 3:
                                    stitch += [(8 + sl, 1) for sl in range(4)]
                        if first:
                            # prefetch phase-3 constants during attention
                            nc.sync.dma_start(wo_s[:], wo.ap().rearrange("(c p) d -> p c d", p=128).bitcast(F32R))
                            nc.sync.dma_start(bo4_s[:], bo4.ap())
                            first = False
                        norm_half(1)
                    # queue this qb-pair's output projection for stitching
                    stitch += [
                        (8 * qbp + sl, qbp)
                        for sl in (range(4, 8) if qbp else range(8))
                    ]
                    if qbp == 1 and dbg:
                        for i in range(2):
                            nc.sync.dma_start(dctx[i, :, :], ctx_s[i][:].bitcast(F32))
                # flush remaining oproj tiles (tail), 2-deep via sc-slot tiles
                for i, (st_, hf_) in enumerate(stitch):
                    emit_po_st(st_, hf_, alt=(i % 2 == 0), tail=True)
                if not os_mod.environ.get("BASS_SIM_NO_RS"):
                    import concourse.mybir as mybir_mod
                    for half in range(2):
                        nc.gpsimd.collective_compute(
                            "ReduceScatter", mybir_mod.AluOpType.add,
                            replica_groups=[[0, 1, 2, 3], [4, 5, 6, 7]],
                            ins=[rs_in[half].opt()], outs=[rs_out[half].opt()],
                        )
                        nc.sync.dma_start(
                            out[256 * half : 256 * (half + 1), :], rs_out[half][:])


    nc.compile()
    return nc


def _prep_inputs(query, key_, value, w_q, b_q, w_k, b_k, w_v, b_v, w_o, b_o):
    """Build the 8 per-core input maps (host-side sharding / re-layout)."""
    import ml_dtypes
    f32 = np.float32
    bf16 = ml_dtypes.bfloat16
    # triangular mask patterns: t in 0..3, allowed iff j >= r + 128*t
    r = np.arange(128)[:, None]
    j = np.arange(512)[None, :]
    masks = np.concatenate(
        [(j >= r + 128 * t).astype(bf16) for t in range(4)], axis=1
    )  # [128, 2048]
    bo4 = (np.asarray(b_o, f32) / 4.0).reshape(1, D).astype(bf16)

    wqT = np.ascontiguousarray(np.asarray(w_q, f32).T)  # [D_in, D_out]
    wkT = np.ascontiguousarray(np.asarray(w_k, f32).T)
    wvT = np.ascontiguousarray(np.asarray(w_v, f32).T)
    woT = np.ascontiguousarray(np.asarray(w_o, f32).T)  # [D_in, D_out]

    xT = {}
    for g in range(B):
        xT[("q", g)] = np.ascontiguousarray(np.asarray(query[g], f32).T.astype(bf16))
        xT[("k", g)] = np.ascontiguousarray(np.asarray(key_[g], f32).T.astype(bf16))
        xT[("v", g)] = np.ascontiguousarray(np.asarray(value[g], f32).T.astype(bf16))

    in_maps = []
    for c in range(N_CORES):
        g, p = c // 4, c % 4
        fsel = slice(FPC * p, FPC * (p + 1))
        in_maps.append({
            "xq": xT[("q", g)],
            "xk": xT[("k", g)],
            "xv": xT[("v", g)],
            "wq": np.ascontiguousarray(wqT[:, fsel].astype(bf16)),
            "wk": np.ascontiguousarray(wkT[:, fsel].astype(bf16)),
            "wv": np.ascontiguousarray(wvT[:, fsel].astype(bf16)),
            "wo": np.ascontiguousarray(woT[fsel, :]),
            "bq": np.ascontiguousarray(np.asarray(b_q, f32)[fsel].reshape(FPC, 1)),
            "bk": np.ascontiguousarray(np.asarray(b_k, f32)[fsel].reshape(FPC, 1)),
            "bv": np.ascontiguousarray(np.asarray(b_v, f32)[fsel].reshape(1, FPC)),
            "bo4": bo4,
            "masks": masks,
        })
    return in_maps


def run(inputs, trace=False):
    from concourse.bass_utils import run_bass_kernel_spmd

    if "nc" not in _CACHE:
        _CACHE["nc"] = _build_nc()
    nc = _CACHE["nc"]
    in_maps = _prep_inputs(
        inputs["query"], inputs["key_"], inputs["value"],
        inputs["w_q"], inputs["b_q"], inputs["w_k"], inputs["b_k"],
        inputs["w_v"], inputs["b_v"], inputs["w_o"], inputs["b_o"],
    )
    res = run_bass_kernel_spmd(
        nc, in_maps, core_ids=list(range(N_CORES)), trace=trace,
    )
    out = np.empty((B, S, D), np.float32)
    for c in range(N_CORES):
        g, p = c // 4, c % 4
        # RS half i scatters q rows [1024*i + 256*p, 1024*i + 256*(p+1))
        out[g, 256 * p : 256 * (p + 1), :] = res.results[c]["out"][0:256]
        out[g, 1024 + 256 * p : 1024 + 256 * (p + 1), :] = res.results[c]["out"][256:512]
    return out, res


def kernel(**inputs):
    out, _ = run(inputs, trace=False)
    return out


# revision 113
# speedup vs baseline: 1.0059x; 1.0059x over previous
"""Causal multi-head attention on 8 Trainium2 NeuronCores.

Sharding: core c -> (batch g = c // 4, head-group p = c % 4, heads 4p..4p+3).
Each core projects Q/K/V for its batch with its 256 feature columns
(column-sharded w_q/w_k/w_v), runs causal attention for its 4 heads in
transposed (scores.T) layout with an augmented-ones column on V to get the
softmax denominators for free, computes the partial output projection with
its 256 rows of w_o, and a ReduceScatter over each batch group sums the
partials and hands every core its own 512-row output shard.

x / w_qkv / probs / V / masks in bf16 (halves DMA + 2x DVE mask-muls);
scores and all PSUM accumulation fp32; qT/kT/ctx/w_o float32r.
V is projected directly into natural [kpos, dim] layout (x chunk as the
stationary operand) so no PE transposes are needed. Right causal edge is
trimmed to 256-col granularity. Output-projection bias adds run on the
(otherwise idle) Pool engine.
"""

import numpy as np

B, S, D, H = 2, 2048, 1024, 16
DK = D // H  # 64
N_CORES = 8
FPC = 256  # features per core

_CACHE = {}

TRIM = (0, 128, 256, 256)  # leading fully-masked cols to skip, by ki%4


def _build_nc():
    import os as os_mod  # noqa: F401  (used in emit_oproj_half closure)
    import concourse.mybir as mybir
    import concourse.tile as tile
    from concourse import bacc

    F32 = mybir.dt.float32
    F32R = mybir.dt.float32r
    BF16 = mybir.dt.bfloat16
    Exp = mybir.ActivationFunctionType.Exp

    nc = bacc.Bacc("TRN2", target_bir_lowering=False, debug=False, num_devices=8)

    xq = nc.dram_tensor("xq", [D, S], BF16, kind="ExternalInput")
    xk = nc.dram_tensor("xk", [D, S], BF16, kind="ExternalInput")
    xv = nc.dram_tensor("xv", [D, S], BF16, kind="ExternalInput")
    wq = nc.dram_tensor("wq", [D, FPC], BF16, kind="ExternalInput")
    wk = nc.dram_tensor("wk", [D, FPC], BF16, kind="ExternalInput")
    wv = nc.dram_tensor("wv", [D, FPC], BF16, kind="ExternalInput")
    wo = nc.dram_tensor("wo", [FPC, D], F32, kind="ExternalInput")
    bq = nc.dram_tensor("bq", [FPC, 1], F32, kind="ExternalInput")
    bk = nc.dram_tensor("bk", [FPC, 1], F32, kind="ExternalInput")
    bv = nc.dram_tensor("bv", [1, FPC], F32, kind="ExternalInput")
    bo4 = nc.dram_tensor("bo4", [1, D], BF16, kind="ExternalInput")
    masks = nc.dram_tensor("masks", [128, 2048], BF16, kind="ExternalInput")
    out = nc.dram_tensor("out", [512, D], F32, kind="ExternalOutput")
    dbg = bool(os_mod.environ.get("BASS_DEBUG_DUMP"))
    if dbg:
        dq = nc.dram_tensor("dq", [2, 128, S], F32, kind="ExternalOutput")
        dk = nc.dram_tensor("dk", [2, 128, S], F32, kind="ExternalOutput")
        dv = nc.dram_tensor("dv", [128, 16, 260], BF16, kind="ExternalOutput")
        dctx = nc.dram_tensor("dctx", [2, 128, S], F32, kind="ExternalOutput")

    NQB = S // 512  # 4 q blocks

    from contextlib import ExitStack
    stack = ExitStack()
    with tile.TileContext(nc) as tc:
        with (
            tc.tile_pool(name="consts", bufs=1) as consts,
            tc.tile_pool(name="persist", bufs=1) as persist,
            tc.tile_pool(name="xin", bufs=6) as xin,
            tc.tile_pool(name="xkres", bufs=8) as xkres,
            tc.tile_pool(name="probs", bufs=4) as probs,
            tc.tile_pool(name="small", bufs=4) as small,
            tc.tile_pool(name="oout", bufs=4) as oout,
            tc.tile_pool(name="dram", bufs=1, space="DRAM") as dram,
        ):
            # ---- constants ----
            wq_s = consts.tile([128, 8, FPC], BF16, tag="wq")
            wk_s = consts.tile([128, 8, FPC], BF16, tag="wk")
            wv_s = consts.tile([128, 8, FPC], BF16, tag="wv")
            wo_s = consts.tile([128, 2, D], F32R, tag="wo")
            masks_s = consts.tile([128, 2048], BF16, tag="masks")
            bq_s = consts.tile([128, 2], F32, tag="bq")
            bk_s = consts.tile([128, 2], F32, tag="bk")
            bv_row = consts.tile([1, FPC], F32, tag="bvr")
            bv_bc = consts.tile([128, FPC], F32, tag="bvb")
            bo4_s = consts.tile([1, D], BF16, tag="bo4")
            ones_r = consts.tile([1, 128], BF16, tag="ones")
            wu_in = consts.tile([128, 64], BF16, tag="wu")

            # ---- persistent activations ----
            qT_s = [persist.tile([128, S], F32R, tag=f"qT{i}", name=f"qT{i}") for i in range(2)]
            kT_s = [persist.tile([128, S], F32R, tag=f"kT{i}", name=f"kT{i}") for i in range(2)]
            # V natural layout: [kpos(128) , s-tile st, head strips of 65 (64 dims + ones)]
            v_s = persist.tile([128, 16, 4 * 65], BF16, tag="v")
            ctx_s = [persist.tile([128, S], F32R, tag=f"ctx{i}", name=f"ctx{i}") for i in range(2)]

            # PE p-state: the cost model runs matmuls at full clock only when
            # their dispatch lands >3us after the engine's current busy-run
            # began. Seed one long busy run with cheap warm-up matmuls and
            # avoid engine idle gaps from here on.
            nc.vector.memset(wu_in[:], 0.5)
            nc.vector.memset(ones_r[:], 1.0)

            # startup DMAs: all on the sync queue so the shared DMA engine
            # serves them in exactly this order (x chunks just-in-time,
            # weights squeezed between)
            nc.sync.dma_start(wv_s[:, 0, :], wv[0:128, :])
            xt0 = xin.tile([128, S], BF16, tag="x", name="xt0")
            for piece in range(4):
                # parallel dispatch queues: each dma_start costs ~650ns of
                # queue-dispatch latency, serial per engine
                (nc.scalar if piece % 2 else nc.sync).dma_start(
                    xt0[:, 512 * piece : 512 * (piece + 1)],
                    xv[0:128, 512 * piece : 512 * (piece + 1)])
            nc.scalar.dma_start(bv_row[:], bv.ap())
            nc.scalar.dma_start(
                wv_s[:, 1:8, :],
                wv[128:1024, :].rearrange("(kc p) f -> p kc f", p=128))
            nc.gpsimd.partition_broadcast(bv_bc[:], bv_row[:])
            # ones columns of V_aug (col 64 of each head's 65-wide strip)
            for h in range(4):
                nc.vector.memset(v_s[:, :, 65 * h + 64 : 65 * h + 65].bitcast(BF16), 1.0)

            # ---- phase 1: projections (shared PSUM pool, slots recycle) ----
            with tc.tile_pool(name="psProj", bufs=8, space="PSUM") as psP:
                # V pass first (direct natural layout: x chunk slices stationary,
                # wv moving). psum tile sp covers s-tiles 2sp, 2sp+1 (256 feats each)
                psv = {sp: psP.tile([128, 512], F32, tag="pp", name=f"psv{sp}")
                       for sp in range(8)}
                def warm(n):  # PE warm-up / bridge runs (overwritten later)
                    for _ in range(n):
                        nc.tensor.matmul(
                            psv[7][0:64, 0:64], wu_in[:, 0:64], wu_in[:, 0:64],
                            start=True, stop=True, skip_group_check=True)
                warm(56)
                # pre-issue all chunk DMAs: the xin pool rotation turns this
                # into a bufs-deep prefetch pipeline
                xts = [xt0]
                for kc in range(1, 8):
                    xt = xin.tile([128, S], BF16, tag="x")
                    nc.sync.dma_start(xt[:], xv[128 * kc : 128 * (kc + 1), :])
                    xts.append(xt)
                for kc in range(8):
                    xt = xts[kc]
                    for sp in range(8):
                        for half in range(2):
                            st = 2 * sp + half
                            # one accumulation group per PSUM bank: start=True
                            # zeroes the whole bank, so only the first write
                            # into the bank may carry it
                            nc.tensor.matmul(
                                psv[sp][:, 256 * half : 256 * (half + 1)],
                                xt[:, 128 * st : 128 * (st + 1)],
                                wv_s[:, kc, :],
                                start=(kc == 0 and half == 0),
                                stop=(kc == 7 and half == 1),
                                skip_group_check=True,
                            )
                    if kc == 3:
                        # prefetch Q weights/bias + first xq chunk during V
                        nc.scalar.dma_start(wq_s[:], wq.ap().rearrange("(kc p) f -> p kc f", p=128))
                        nc.scalar.dma_start(bq_s[:], bq.ap().rearrange("(t p) o -> p (t o)", p=128))
                    elif kc == 5:
                        xq0 = xin.tile([128, S], BF16, tag="x", name="xq0")
                        nc.sync.dma_start(xq0[:], xq[0:128, :])
                    elif kc == 6:
                        nc.scalar.dma_start(wk_s[:], wk.ap().rearrange("(kc p) f -> p kc f", p=128))
                        nc.scalar.dma_start(bk_s[:], bk.ap().rearrange("(t p) o -> p (t o)", p=128))
                        nc.scalar.dma_start(masks_s[:], masks.ap())
                for sp in range(8):
                    for half in range(2):
                        st = 2 * sp + half
                        dst = v_s[:, st, :].rearrange("p (h x) -> p h x", x=65)[:, :, 0:64]
                        nc.vector.tensor_add(
                            dst,
                            psv[sp][:, 256 * half : 256 * (half + 1)].rearrange(
                                "p (h x) -> p h x", x=64),
                            bv_bc[:].rearrange("p (h x) -> p h x", x=64),
                        )

                # Q pass: psum [2pt x 4qb] accumulate over 8 k-chunks
                psq = {(pt, qb): psP.tile([128, 512], F32, tag="pp", name=f"psq{pt}{qb}")
                       for pt in range(2) for qb in range(NQB)}
                xts = [xq0]
                for kc in range(1, 8):
                    xt = xin.tile([128, S], BF16, tag="x")
                    nc.sync.dma_start(xt[:], xq[128 * kc : 128 * (kc + 1), :])
                    xts.append(xt)
                for kc in range(8):
                    xt = xts[kc]
                    for pt in range(2):
                        for qb in range(NQB):
                            nc.tensor.matmul(
                                psq[(pt, qb)][:],
                                wq_s[:, kc, 128 * pt : 128 * (pt + 1)],
                                xt[:, 512 * qb : 512 * (qb + 1)],
                                start=(kc == 0), stop=(kc == 7),
                            )
                    if kc == 5:
                        xk0 = xkres.tile([128, S], BF16, tag="xk", name="xk0")
                        nc.sync.dma_start(xk0[:], xk[0:128, :])
                for pt in range(2):
                    for qb in range(NQB):
                        nc.vector.tensor_scalar_add(
                            qT_s[pt][:, 512 * qb : 512 * (qb + 1)],
                            psq[(pt, qb)][:], bq_s[:, pt : pt + 1],
                        )
                # K pass
                psk = {(0, qb): psP.tile([128, 512], F32, tag="pp", name=f"psk0{qb}")
                       for qb in range(NQB)}
                xts = [xk0]
                for kc in range(1, 8):
                    xt = xkres.tile([128, S], BF16, tag="xk")
                    nc.sync.dma_start(xt[:], xk[128 * kc : 128 * (kc + 1), :])
                    xts.append(xt)
                xk_res = xts
                for kc in range(8):
                    xt = xts[kc]
                    for qb in range(NQB):
                        nc.tensor.matmul(
                            psk[(0, qb)][:],
                            wk_s[:, kc, 0:128],
                            xt[:, 512 * qb : 512 * (qb + 1)],
                            start=(kc == 0), stop=(kc == 7),
                        )

                # spread the last copies across 3 engines: the attention
                # PSUM pools can only open once every psProj tile is released
                for qb in range(NQB):
                    dst = kT_s[0][:, 512 * qb : 512 * (qb + 1)]
                    if qb % 2 == 1:
                        nc.scalar.activation(
                            out=dst, in_=psk[(0, qb)][:],
                            func=mybir.ActivationFunctionType.Identity,
                            bias=bk_s[:, 0:1])
                    else:
                        nc.vector.tensor_scalar_add(
                            dst, psk[(0, qb)][:], bk_s[:, 0:1])

            if dbg:
                for i in range(2):
                    nc.sync.dma_start(dq[i, :, :], qT_s[i][:].bitcast(F32))
                    nc.sync.dma_start(dk[i, :, :], kT_s[i][:].bitcast(F32))
                nc.sync.dma_start(dv[:, :, :], v_s[:])

            # ---- output projection halves + split ReduceScatter ----
            rs_in = [dram.tile([S // 2, D], F32, name=f"rs_in{i}") for i in range(2)]
            rs_out = [dram.tile([256, D], F32, name=f"rs_out{i}") for i in range(2)]

            def emit_oproj_half(half, psO):
                # q rows [1024*half, 1024*half+1024) = ctx_s cols of qb-pair `half`
                # bias folded in via a ones-row matmul; rs_in DMA'd straight
                # from PSUM
                for sl in range(8):
                    st = 8 * half + sl
                    po = psO.tile([128, 2, 512], F32, tag="po", bufs=4, name="po")
                    for nb in range(2):
                        for fc in range(2):
                            nc.tensor.matmul(
                                po[:, nb, :],
                                ctx_s[fc][:, 128 * st : 128 * (st + 1)],
                                wo_s[:, fc, 512 * nb : 512 * (nb + 1)],
                                start=(fc == 0), stop=False,
                                skip_group_check=True,
                            )
                        nc.tensor.matmul(
                            po[:, nb, :],
                            ones_r[:],
                            bo4_s[:, 512 * nb : 512 * (nb + 1)],
                            start=False, stop=True,
                            skip_group_check=True,
                        )
                    ot = oout.tile([128, D], F32, tag="ot")
                    if sl % 2 == 0:
                        nc.scalar.activation(
                            out=ot[:].rearrange("p (n x) -> p n x", n=2), in_=po[:],
                            func=mybir.ActivationFunctionType.Copy)
                    else:
                        nc.vector.tensor_copy(
                            ot[:].rearrange("p (n x) -> p n x", n=2), po[:])
                    nc.sync.dma_start(rs_in[half][128 * sl : 128 * (sl + 1), :], ot[:])
                if not os_mod.environ.get("BASS_SIM_NO_RS"):
                    import concourse.mybir as mybir_mod
                    nc.gpsimd.collective_compute(
                        "ReduceScatter", mybir_mod.AluOpType.add,
                        replica_groups=[[0, 1, 2, 3], [4, 5, 6, 7]],
                        ins=[rs_in[half].opt()], outs=[rs_out[half].opt()],
                    )
                    nc.sync.dma_start(
                        out[256 * half : 256 * (half + 1), :], rs_out[half][:])
                else:
                    nc.sync.dma_start(
                        out[256 * half : 256 * (half + 1), :],
                        rs_in[half][0:256, :])

            # ---- phase 2: attention ----
            # One head-stream at a time; PSUM split 4/2/2 banks:
            #   psS: sc scores [128,1024] x2 bufs (4 banks)
            #   psA: ctx accumulators [65,1024] x1 buf (2 banks)
            #   psO: oproj po [128,2,512] x1 buf (2 banks)
            # A depth-2 software pipeline (ctx matmuls for step ki-2 emitted
            # after the score matmuls for step ki) hides both the exp/mask
            # chain and the ctx-slot norm drain at head seams. Output
            # projection tiles are stitched one-at-a-time into the next
            # qb-pair's (Act-bound) attention loop.
            first = True
            with (
                tc.tile_pool(name="rbcp", bufs=2) as rbcp,
                tc.tile_pool(name="psS", bufs=1, space="PSUM") as psS,
                tc.tile_pool(name="psA", bufs=1, space="PSUM") as psA,
                tc.tile_pool(name="psO", bufs=1, space="PSUM") as psO,
            ):
                def emit_po_st(st, half, alt=False, tail=False):
                    if alt:
                        pot = psS.tile([128, 1024], F32, tag="sc", bufs=2, name="sc")
                        po = pot[:].rearrange("p (n x) -> p n x", n=2)
                    else:
                        pot = psO.tile([128, 2, 512], F32, tag="po", bufs=1, name="po")
                        po = pot[:]
                    for nb in range(2):
                        for fc in range(2):
                            nc.tensor.matmul(
                                po[:, nb, :],
                                ctx_s[fc][:, 128 * st : 128 * (st + 1)],
                                wo_s[:, fc, 512 * nb : 512 * (nb + 1)],
                                start=(fc == 0), stop=False,
                                skip_group_check=True,
                            )
                        nc.tensor.matmul(
                            po[:, nb, :], ones_r[:],
                            bo4_s[:, 512 * nb : 512 * (nb + 1)],
                            start=False, stop=True,
                            skip_group_check=True,
                        )
                    ot = oout.tile([128, D], F32, tag="ot")
                    otv = ot[:].rearrange("p (n x) -> p n x", n=2)
                    if alt or tail:
                        # tail: split the copy across Act+DVE (halves latency)
                        nc.scalar.activation(
                            out=otv[:, 0, :], in_=po[:, 0, :],
                            func=mybir.ActivationFunctionType.Copy)
                        nc.vector.tensor_copy(otv[:, 1, :], po[:, 1, :])
                    else:
                        # stitched into Act-bound attention: keep Act free
                        nc.vector.tensor_copy(otv, po[:])
                    sl = st % 8
                    nc.sync.dma_start(rs_in[half][128 * sl : 128 * (sl + 1), :], ot[:])
                    if sl < 2 and os_mod.environ.get("BASS_SIM_NO_RS"):
                        # NO_RS stub: emit the out rows straight from SBUF so
                        # they don't queue behind the tail rs_in writes
                        nc.sync.dma_start(
                            out[256 * half + 128 * sl : 256 * half + 128 * (sl + 1), :],
                            ot[:])

                stitch = []
                # K projection for heads 2-3 (kT_s[1]) runs inside the
                # Act-bound qbp0 attention, in borrowed oproj PSUM banks
                kbs = {}

                def kpt1_thunk(pair, half):
                    def f():
                        if pair not in kbs:
                            kbs[pair] = psO.tile([128, 2, 512], F32, tag="po",
                                                 bufs=1, name="po")
                        kb = kbs[pair]
                        for kc in range(4 * half, 4 * half + 4):
                            for j in range(2):
                                qb = 2 * pair + j
                                nc.tensor.matmul(
                                    kb[:, j, :], wk_s[:, kc, 128:256],
                                    xk_res[kc][:, 512 * qb : 512 * (qb + 1)],
                                    start=(kc == 0), stop=(kc == 7),
                                    skip_group_check=True)
                        if half == 1:
                            for j in range(2):
                                qb = 2 * pair + j
                                dst = kT_s[1][:, 512 * qb : 512 * (qb + 1)]
                                if j:
                                    nc.scalar.activation(
                                        out=dst, in_=kb[:, j, :],
                                        func=mybir.ActivationFunctionType.Identity,
                                        bias=bk_s[:, 1:2])
                                else:
                                    nc.vector.tensor_scalar_add(
                                        dst, kb[:, j, :], bk_s[:, 1:2])
                    return f

                bgwork = [kpt1_thunk(p, hf) for p in (0, 1) for hf in (0, 1)]
                for qbp in range(2):
                    nkt = 8 * qbp + 8  # k-tiles needed by this qb pair
                    q0, q1 = 2 * qbp, 2 * qbp + 1
                    step = 0
                    for h in range(4):
                        pt, row = h // 2, 64 * (h % 2)
                        qT_h = qT_s[pt][row : row + 64, :]
                        kT_h = kT_s[pt][row : row + 64, :]
                        ctx_t = psA.tile([65, 1024], F32, tag="ctx", bufs=1, name="ctx")

                        def norm_half(hf, qbp_=qbp, pt=pt, row=row, ctx_t=ctx_t):
                            cs = slice(512 * hf, 512 * (hf + 1))
                            recip = small.tile([1, 512], F32, tag="recip")
                            nc.vector.reciprocal(recip[:], ctx_t[64:65, cs])
                            rbc = rbcp.tile([64, 512], F32, tag="rbc", bufs=6)
                            nc.gpsimd.partition_broadcast(rbc[:], recip[:])
                            nc.vector.tensor_mul(
                                ctx_s[pt][row : row + 64,
                                          1024 * qbp_ + 512 * hf : 1024 * qbp_ + 512 * (hf + 1)],
                                ctx_t[0:64, cs], rbc[:],
                            )

                        pend = []

                        def pop_pend():
                            group, lo_stop = pend.pop(0)
                            for dst, vtl, src_, st_, sp_ in group:
                                nc.tensor.matmul(dst, vtl, src_, start=st_, stop=sp_,
                                                 skip_group_check=True)
                            return lo_stop

                        # single-qb tail k-tiles are packed two per sc tile
                        # (one exp instead of two, halves the step count)
                        items = ([[ki] for ki in range(min(nkt, 4 * q1))]
                                 + [[4 * q1, 4 * q1 + 1], [4 * q1 + 2, 4 * q1 + 3]])
                        step_in_head = 0
                        for item in items:
                            if qbp == 0 and step_in_head % 3 == 2 and not bgwork:
                                sc = psO.tile([128, 2, 512], F32, tag="po", bufs=1,
                                              name="po")[:].rearrange("p n x -> p (n x)")
                            else:
                                sc = psS.tile([128, 1024], F32, tag="sc", bufs=2, name="sc")
                            pr = probs.tile([128, 1024], BF16, tag="pr", bufs=8, name="pr")
                            paired = len(item) > 1
                            infos = []
                            exp_lo = None
                            for j, ki in enumerate(item):
                                dqb = ki // 4  # diagonal q block (absolute)
                                has_diag = dqb >= q0
                                qlo = max(dqb, q0)
                                off = TRIM[ki % 4] if has_diag else 0
                                infos.append((j, ki, has_diag, qlo, off))
                                # matmul writes stay within one PSUM bank
                                # (512 f32): one matmul per 512-col q block.
                                # paired k-tiles write the full bank so the
                                # shared exp never reads unwritten PSUM
                                for qb in range(qlo, q1 + 1):
                                    o = 0 if paired else (off if qb == qlo else 0)
                                    r = (512 * j if paired
                                         else 512 * (qb - 2 * qbp)) + o
                                    nc.tensor.matmul(
                                        sc[:, r : (r - o) + 512],
                                        kT_h[:, 128 * ki : 128 * (ki + 1)],
                                        qT_h[:, 512 * qb + o : 512 * (qb + 1)],
                                        start=True, stop=True,
                                    )
                                    if j == 0 and qb == qlo:
                                        exp_lo = r
                            nc.scalar.activation(
                                out=pr[:, exp_lo:1024], in_=sc[:, exp_lo:1024],
                                func=Exp, scale=0.125,
                            )
                            cur = []
                            lo_stop = False
                            for j, ki, has_diag, qlo, off in infos:
                                vt = v_s[:, ki, 65 * h : 65 * h + 65]
                                prm = None
                                if has_diag:
                                    pcol = (512 * j if paired
                                            else 512 * (qlo - 2 * qbp)) + off
                                    prm = probs.tile([128, 512], BF16, tag="prm", bufs=8, name="prm")
                                    nc.vector.tensor_mul(
                                        prm[:, 0 : 512 - off], pr[:, pcol : (pcol - off) + 512],
                                        masks_s[:, 512 * (ki % 4) + off : 512 * (ki % 4) + 512],
                                    )
                                for qb in range(qlo, q1 + 1):
                                    o = off if qb == qlo else 0
                                    pcol = (512 * j if paired
                                            else 512 * (qb - 2 * qbp)) + o
                                    ccol = 512 * (qb - 2 * qbp) + o
                                    if has_diag and qb == qlo:
                                        rhs = prm[:, 0 : 512 - o]
                                    else:
                                        rhs = pr[:, pcol : (pcol - o) + 512]
                                    cur.append((
                                        ctx_t[:, ccol : (ccol - o) + 512],
                                        vt, rhs,
                                        ki == 0, ki == 4 * qb + 3))
                                    if ki == 8 * qbp + 3 and qb == 2 * qbp:
                                        lo_stop = True
                            pend.append((cur, lo_stop))
                            depth = 2
                            while len(pend) > depth:
                                if pop_pend():
                                    # cols [0:512] of ctx are complete:
                                    # normalize the first half early
                                    norm_half(0)
                                    if qbp == 1 and h == 3:
                                        # oproj1 tiles reading half-0 cols are
                                        # now fully normalized: stitch them in
                                        stitch += [(8 + sl, 1) for sl in range(4)]
                            step += 1
                            step_in_head += 1
                            if bgwork and qbp == 0 and step % 2 == 1:
                                bgwork.pop(0)()
                            if stitch and (step % 6 == 5
                                           or (qbp == 1 and h == 3
                                               and step_in_head >= 12)):
                                st_, hf_ = stitch.pop(0)
                                emit_po_st(st_, hf_)
                        while pend:
                            if pop_pend():
                                norm_half(0)
                                if qbp == 1 and h == 3:
                                    stitch += [(8 + sl, 1) for sl in range(4)]
                        if first:
                            # prefetch phase-3 constants during attention
                            nc.sync.dma_start(wo_s[:], wo.ap().rearrange("(c p) d -> p c d", p=128).bitcast(F32R))
                            nc.sync.dma_start(bo4_s[:], bo4.ap())
                            first = False
                        norm_half(1)
                    # queue this qb-pair's output projection for stitching
                    stitch += [
                        (8 * qbp + sl, qbp)
                        for sl in (range(4, 8) if qbp else range(8))
                    ]
                    if qbp == 1 and dbg:
                        for i in range(2):
                            nc.sync.dma_start(dctx[i, :, :], ctx_s[i][:].bitcast(F32))
                # flush remaining oproj tiles (tail), 2-deep via sc-slot tiles
                for i, (st_, hf_) in enumerate(stitch):
                    emit_po_st(st_, hf_, alt=(i % 2 == 0), tail=True)
                if not os_mod.environ.get("BASS_SIM_NO_RS"):
                    import concourse.mybir as mybir_mod
                    for half in range(2):
                        nc.gpsimd.collective_compute(
                            "ReduceScatter", mybir_mod.AluOpType.add,
                            replica_groups=[[0, 1, 2, 3], [4, 5, 6, 7]],
                            ins=[rs_in[half].opt()], outs=[rs_out[half].opt()],
                        )
                        nc.sync.dma_start(
                            out[256 * half : 256 * (half + 1), :], rs_out[half][:])


    nc.compile()
    return nc


def _prep_inputs(query, key_, value, w_q, b_q, w_k, b_k, w_v, b_v, w_o, b_o):
    """Build the 8 per-core input maps (host-side sharding / re-layout)."""
    import ml_dtypes
    f32 = np.float32
    bf16 = ml_dtypes.bfloat16
    # triangular mask patterns: t in 0..3, allowed iff j >= r + 128*t
    r = np.arange(128)[:, None]
    j = np.arange(512)[None, :]
    masks = np.concatenate(
        [(j >= r + 128 * t).astype(bf16) for t in range(4)], axis=1
    )  # [128, 2048]
    bo4 = (np.asarray(b_o, f32) / 4.0).reshape(1, D).astype(bf16)

    wqT = np.ascontiguousarray(np.asarray(w_q, f32).T)  # [D_in, D_out]
    wkT = np.ascontiguousarray(np.asarray(w_k, f32).T)
    wvT = np.ascontiguousarray(np.asarray(w_v, f32).T)
    woT = np.ascontiguousarray(np.asarray(w_o, f32).T)  # [D_in, D_out]

    xT = {}
    for g in range(B):
        xT[("q", g)] = np.ascontiguousarray(np.asarray(query[g], f32).T.astype(bf16))
        xT[("k", g)] = np.ascontiguousarray(np.asarray(key_[g], f32).T.astype(bf16))
        xT[("v", g)] = np.ascontiguousarray(np.asarray(value[g], f32).T.astype(bf16))

    in_maps = []
    for c in range(N_CORES):
        g, p = c // 4, c % 4
        fsel = slice(FPC * p, FPC * (p + 1))
        in_maps.append({
            "xq": xT[("q", g)],
            "xk": xT[("k", g)],
            "xv": xT[("v", g)],
            "wq": np.ascontiguousarray(wqT[:, fsel].astype(bf16)),
            "wk": np.ascontiguousarray(wkT[:, fsel].astype(bf16)),
            "wv": np.ascontiguousarray(wvT[:, fsel].astype(bf16)),
            "wo": np.ascontiguousarray(woT[fsel, :]),
            "bq": np.ascontiguousarray(np.asarray(b_q, f32)[fsel].reshape(FPC, 1)),
            "bk": np.ascontiguousarray(np.asarray(b_k, f32)[fsel].reshape(FPC, 1)),
            "bv": np.ascontiguousarray(np.asarray(b_v, f32)[fsel].reshape(1, FPC)),
            "bo4": bo4,
            "masks": masks,
        })
    return in_maps


def run(inputs, trace=False):
    from concourse.bass_utils import run_bass_kernel_spmd

    if "nc" not in _CACHE:
        _CACHE["nc"] = _build_nc()
    nc = _CACHE["nc"]
    in_maps = _prep_inputs(
        inputs["query"], inputs["key_"], inputs["value"],
        inputs["w_q"], inputs["b_q"], inputs["w_k"], inputs["b_k"],
        inputs["w_v"], inputs["b_v"], inputs["w_o"], inputs["b_o"],
    )
    res = run_bass_kernel_spmd(
        nc, in_maps, core_ids=list(range(N_CORES)), trace=trace,
    )
    out = np.empty((B, S, D), np.float32)
    for c in range(N_CORES):
        g, p = c // 4, c % 4
        # RS half i scatters q rows [1024*i + 256*p, 1024*i + 256*(p+1))
        out[g, 256 * p : 256 * (p + 1), :] = res.results[c]["out"][0:256]
        out[g, 1024 + 256 * p : 1024 + 256 * (p + 1), :] = res.results[c]["out"][256:512]
    return out, res


def kernel(**inputs):
    out, _ = run(inputs, trace=False)
    return out
